# revision 7
# baseline (speedup 1.0000x reference)
"""8-layer GRU (H=1000, batch=1, single step) on 8 trn2 NeuronCores.

Strategy: tensor-parallel row sharding. Core c owns hidden units
[c*125, (c+1)*125) of every gate of every layer.  The W_hh half of the
work (gh_l = W_hh[l] @ h_l) depends only on the *input* hidden states,
so it streams/computes fully in parallel; the W_ih half chains through
layers (gi_l = W_ih[l] @ h'_{l-1}) and needs the full h'_{l-1} on every
core, which is re-assembled with a small per-layer AllGather (500B per
core).  A dummy AllGather issued at kernel start absorbs the expensive
first-collective rendezvous while weights stream.

All weights are pre-transposed and pre-sharded on the host so that each
per-layer weight tile DMAs contiguously into SBUF in the exact lhsT
layout the PE wants: tile[p, kc*375 + g*125 + u] = W[g*1000 + c*125 + u,
kc*125 + p].  All 15 big tiles (7 W_ih + 8 W_hh, 1.5MB each) stay
resident in SBUF (~180KB/partition).

Matvec orientation: out[125,1] per gate accumulated over 8 k-chunks:
lhsT = weight tile slice [125,125], rhs = h as [125,8] (tile[p,kc] =
h[kc*125+p]).  The gathered h' arrives rank-major ([8,125] natural), a
PE transpose via an identity matrix converts it to rhs layout.
"""
import numpy as np

from concourse import bass, mybir, tile
from concourse.bass_utils import run_bass_kernel_spmd

N_CORES = 8
L = 8          # layers
H = 1000
S = 125        # hidden shard per core
KC = 8         # k chunks (contraction split)
G = 3          # gates r,z,n
FP = mybir.dt.float32

# ---------------------------------------------------------------------------
# workaround: this container's walrus build rejects >1 sync wait per
# instruction; split extras onto same-engine carrier NOPs.
MAX_WAITS_PER_INST = 1


def _split_multi_waits(nc, max_waits: int = MAX_WAITS_PER_INST) -> int:
    n_split = 0
    for f in nc.m.functions:
        for bb in f.blocks:
            insts = bb.instructions
            out = []
            for inst in insts:
                si = inst.sync_info
                if si is not None and len(si.on_wait) > max_waits:
                    waits = list(si.on_wait)
                    extra, keep = waits[:-max_waits], waits[-max_waits:]
                    for w in extra:
                        n_split += 1
                        nop = mybir.InstNoOp(
                            name=f"waitsplit-{n_split}-{inst.name}",
                            sync_info=mybir.SyncInfo(on_wait=[w], on_update=[]),
                            bass_nofuse=True,
                            engine=inst.engine,
                        )
                        out.append(nop)
                    si.on_wait = keep
                out.append(inst)
            insts[:] = out
    return n_split


# ---------------------------------------------------------------------------
def build_kernel() -> bass.Bass:
    nc = bass.Bass()

    # ---- per-core inputs (host pre-sharded / pre-transposed) ----
    wt0 = nc.declare_dram_parameter("wt0", [2, G * S], FP, isOutput=False)
    wtih = nc.declare_dram_parameter("wtih", [L - 1, S, KC * G * S], FP, isOutput=False)
    wthh = nc.declare_dram_parameter("wthh", [L, S, KC * G * S], FP, isOutput=False)
    hrhs = nc.declare_dram_parameter("hrhs", [S, L * KC], FP, isOutput=False)
    hown = nc.declare_dram_parameter("hown", [S, L], FP, isOutput=False)
    brz = nc.declare_dram_parameter("brz", [S, L * 2], FP, isOutput=False)
    bin_ = nc.declare_dram_parameter("bin", [S, L], FP, isOutput=False)
    bhn = nc.declare_dram_parameter("bhn", [S, L], FP, isOutput=False)
    x2 = nc.declare_dram_parameter("x2", [2, 1], FP, isOutput=False)
    wout = nc.declare_dram_parameter("wout", [S, KC], FP, isOutput=False)
    bout = nc.declare_dram_parameter("bout", [1, 1], FP, isOutput=False)
    id8 = nc.declare_dram_parameter("id8", [KC, KC], FP, isOutput=False)

    hid_out = nc.declare_dram_parameter("hid_out", [L, H], FP, isOutput=True)
    flow_out = nc.declare_dram_parameter("flow_out", [1, 1], FP, isOutput=True)

    with tile.TileContext(nc) as tc:
        with (
            tc.tile_pool(name="dram", bufs=1, space="DRAM") as dram,
            tc.tile_pool(name="wp", bufs=1) as wp,
            tc.tile_pool(name="small", bufs=1) as small,
            tc.tile_pool(name="tmp", bufs=3) as tmp,
            tc.tile_pool(name="ps", bufs=2, space="PSUM") as ps,
        ):
            # --- dummy collective to absorb first-collective rendezvous ---
            dummy_in = dram.tile([8, 1], FP, tag="dummy_in")
            dummy_out = dram.tile([64, 1], FP, tag="dummy_out")
            nc.gpsimd.collective_compute(
                "AllGather",
                mybir.AluOpType.bypass,
                replica_groups=[list(range(N_CORES))],
                ins=[dummy_in.opt()],
                outs=[dummy_out.opt()],
            )

            # --- small loads (sync engine HWDGE) ---
            x_sb = small.tile([2, 1], FP, tag="x")
            nc.sync.dma_start(out=x_sb[:], in_=x2[:])
            wt0_sb = small.tile([2, G * S], FP, tag="wt0")
            nc.sync.dma_start(out=wt0_sb[:], in_=wt0[:])
            hrhs_sb = small.tile([S, L * KC], FP, tag="hrhs")
            nc.sync.dma_start(out=hrhs_sb[:], in_=hrhs[:])
            hown_sb = small.tile([S, L], FP, tag="hown")
            nc.sync.dma_start(out=hown_sb[:], in_=hown[:])
            brz_sb = small.tile([S, L * 2], FP, tag="brz")
            nc.sync.dma_start(out=brz_sb[:], in_=brz[:])
            bin_sb = small.tile([S, L], FP, tag="bin")
            nc.sync.dma_start(out=bin_sb[:], in_=bin_[:])
            bhn_sb = small.tile([S, L], FP, tag="bhn")
            nc.sync.dma_start(out=bhn_sb[:], in_=bhn[:])
            wout_sb = small.tile([S, KC], FP, tag="wout")
            nc.sync.dma_start(out=wout_sb[:], in_=wout[:])
            bout_sb = small.tile([1, 1], FP, tag="bout")
            nc.sync.dma_start(out=bout_sb[:], in_=bout[:])
            id8_sb = small.tile([KC, KC], FP, tag="id8")
            nc.sync.dma_start(out=id8_sb[:], in_=id8[:])
            ones_sb = small.tile([S, 1], FP, tag="ones")
            nc.vector.memset(ones_sb[:], 1.0)

            # --- weight DMAs, in chain-consumption order ---
            wtih_sb = [None] * L
            wthh_sb = [None] * L
            wthh_sb[0] = wp.tile([S, KC * G * S], FP, tag="whh0", name="whh0")
            nc.sync.dma_start(out=wthh_sb[0][:], in_=wthh[0])
            for l in range(1, L):
                wtih_sb[l] = wp.tile([S, KC * G * S], FP, tag=f"wih{l}", name=f"wih{l}")
                nc.sync.dma_start(out=wtih_sb[l][:], in_=wtih[l - 1])
                wthh_sb[l] = wp.tile([S, KC * G * S], FP, tag=f"whh{l}", name=f"whh{l}")
                nc.sync.dma_start(out=wthh_sb[l][:], in_=wthh[l])

            # --- per-layer state ---
            hprhs = [None] * L   # gathered h' in rhs layout [S, KC]
            ag_out = [None] * L

            for l in range(L):
                # ---- gi matmuls -> gip [S, G] psum ----
                gip = ps.tile([S, G], FP, tag="gip")
                if l == 0:
                    for g in range(G):
                        nc.tensor.matmul(
                            gip[:, g : g + 1],
                            wt0_sb[:, g * S : (g + 1) * S],
                            x_sb[:],
                            start=True,
                            stop=True,
                        )
                else:
                    rhs = hprhs[l - 1]
                    for g in range(G):
                        for kc in range(KC):
                            nc.tensor.matmul(
                                gip[:, g : g + 1],
                                wtih_sb[l][:, kc * G * S + g * S : kc * G * S + (g + 1) * S],
                                rhs[:, kc : kc + 1],
                                start=(kc == 0),
                                stop=(kc == KC - 1),
                            )

                # ---- gh matmuls -> ghp [S, G] psum ----
                ghp = ps.tile([S, G], FP, tag="ghp")
                for g in range(G):
                    for kc in range(KC):
                        nc.tensor.matmul(
                            ghp[:, g : g + 1],
                            wthh_sb[l][:, kc * G * S + g * S : kc * G * S + (g + 1) * S],
                            hrhs_sb[:, l * KC + kc : l * KC + kc + 1],
                            start=(kc == 0),
                            stop=(kc == KC - 1),
                        )

                # ---- gate math (all [S,1]) ----
                # DVE can read only one PSUM operand: stage gh in SBUF.
                ghsb = tmp.tile([S, G], FP, tag="ghsb")
                nc.scalar.copy(out=ghsb[:], in_=ghp[:])
                # r = sigmoid(gi_r + gh_r + b_rz[2l])
                t_r = tmp.tile([S, 1], FP, tag="t_r")
                nc.vector.tensor_add(t_r[:], gip[:, 0:1], ghsb[:, 0:1])
                r_sb = tmp.tile([S, 1], FP, tag="r")
                nc.scalar.activation(
                    r_sb[:], t_r[:], mybir.ActivationFunctionType.Sigmoid,
                    bias=brz_sb[:, 2 * l : 2 * l + 1],
                )
                # z = sigmoid(gi_z + gh_z + b_rz[2l+1])
                t_z = tmp.tile([S, 1], FP, tag="t_z")
                nc.vector.tensor_add(t_z[:], gip[:, 1:2], ghsb[:, 1:2])
                z_sb = tmp.tile([S, 1], FP, tag="z")
                nc.scalar.activation(
                    z_sb[:], t_z[:], mybir.ActivationFunctionType.Sigmoid,
                    bias=brz_sb[:, 2 * l + 1 : 2 * l + 2],
                )
                # n = tanh(gi_n + b_in + r * (gh_n + b_hn))
                hn = tmp.tile([S, 1], FP, tag="hn")
                nc.vector.tensor_add(hn[:], ghsb[:, 2:3], bhn_sb[:, l : l + 1])
                rhn = tmp.tile([S, 1], FP, tag="rhn")
                nc.vector.tensor_mul(rhn[:], r_sb[:], hn[:])
                s_n = tmp.tile([S, 1], FP, tag="s_n")
                nc.vector.tensor_add(s_n[:], gip[:, 2:3], rhn[:])
                n_sb = tmp.tile([S, 1], FP, tag="n")
                nc.scalar.activation(
                    n_sb[:], s_n[:], mybir.ActivationFunctionType.Tanh,
                    bias=bin_sb[:, l : l + 1],
                )
                # h' = n + z * (h - n)
                d_sb = tmp.tile([S, 1], FP, tag="d")
                nc.vector.tensor_sub(d_sb[:], hown_sb[:, l : l + 1], n_sb[:])
                zd = tmp.tile([S, 1], FP, tag="zd")
                nc.vector.tensor_mul(zd[:], z_sb[:], d_sb[:])
                hp_sb = tmp.tile([S, 1], FP, tag="hp")
                nc.vector.tensor_add(hp_sb[:], zd[:], n_sb[:])

                # ---- exchange: shard -> DRAM -> AllGather -> SBUF ----
                ag_in = dram.tile([S, 1], FP, tag=f"agin{l}")
                nc.gpsimd.dma_start(out=ag_in[:], in_=hp_sb[:])
                ag_out[l] = dram.tile([KC, S], FP, tag=f"agout{l}", name=f"agout{l}")
                nc.gpsimd.collective_compute(
                    "AllGather",
                    mybir.AluOpType.bypass,
                    replica_groups=[list(range(N_CORES))],
                    ins=[ag_in.opt()],
                    outs=[ag_out[l].opt()],
                )
                if l < L - 1 or True:
                    hpnat = tmp.tile([KC, S], FP, tag="hpnat")
                    nc.gpsimd.dma_start(out=hpnat[:], in_=ag_out[l][:])
                    trp = ps.tile([S, KC], FP, tag="trp")
                    nc.tensor.matmul(
                        trp[:], hpnat[:], id8_sb[:], start=True, stop=True
                    )
                    hprhs[l] = small.tile([S, KC], FP, tag=f"hprhs{l}", name=f"hprhs{l}")
                    nc.scalar.copy(out=hprhs[l][:], in_=trp[:])

            # ---- flow = W_out @ h'_7 + b_out ----
            prod = tmp.tile([S, KC], FP, tag="prod")
            nc.vector.tensor_mul(prod[:], wout_sb[:], hprhs[L - 1][:])
            psum_col = tmp.tile([S, 1], FP, tag="pcol")
            nc.vector.tensor_reduce(
                out=psum_col[:], in_=prod[:], op=mybir.AluOpType.add,
                axis=mybir.AxisListType.X,
            )
            flp = ps.tile([1, 1], FP, tag="flp")
            nc.tensor.matmul(flp[:], psum_col[:], ones_sb[:], start=True, stop=True)
            flow_sb = tmp.tile([1, 1], FP, tag="flow")
            nc.vector.tensor_add(flow_sb[:], flp[:], bout_sb[0:1, 0:1])
            nc.sync.dma_start(out=flow_out[:], in_=flow_sb[:])

            # ---- store gathered hiddens (off critical path) ----
            for l in range(L):
                nc.sync.dma_start(
                    out=hid_out[l : l + 1, :],
                    in_=ag_out[l].rearrange("a b -> (a b)").rearrange("(o ab) -> o ab", o=1),
                )

    _split_multi_waits(nc)
    return nc


# ---------------------------------------------------------------------------
def _prep_core_inputs(c, input, hiddens, W_ih0, W_ih, W_hh, b_ih, b_hh, W_out, b_out):
    """Host-side shard/transpose for core c. All fp32."""
    f32 = np.float32
    rows = np.concatenate([np.arange(g * H + c * S, g * H + (c + 1) * S) for g in range(G)])

    def wtile(W):  # W [3H, H] -> [S, KC*G*S]
        R = W[rows, :]                        # [G*S, H]
        T = np.ascontiguousarray(R.T)         # [H, G*S]
        A = T.reshape(KC, S, G * S).transpose(1, 0, 2)  # [S, KC, G*S]
        return np.ascontiguousarray(A.reshape(S, KC * G * S), dtype=f32)

    wt0 = np.ascontiguousarray(W_ih0[rows, :].T, dtype=f32)        # [2, G*S]
    wtih = np.stack([wtile(W_ih[i]) for i in range(L - 1)])        # [7, S, KC*G*S]
    wthh = np.stack([wtile(W_hh[i]) for i in range(L)])            # [8, S, KC*G*S]

    hmat = hiddens[:, 0, :]                                        # [L, H]
    # hrhs[p, l*KC+kc] = h_l[kc*S+p]
    hrhs = np.ascontiguousarray(
        hmat.reshape(L, KC, S).transpose(2, 0, 1).reshape(S, L * KC), dtype=f32
    )
    hown = np.ascontiguousarray(hmat[:, c * S : (c + 1) * S].T, dtype=f32)  # [S, L]

    bsum = b_ih + b_hh                                             # [L, 3H]
    brz = np.empty((S, L * 2), f32)
    for l in range(L):
        brz[:, 2 * l] = bsum[l, 0 * H + c * S : 0 * H + (c + 1) * S]
        brz[:, 2 * l + 1] = bsum[l, 1 * H + c * S : 1 * H + (c + 1) * S]
    bin_ = np.ascontiguousarray(b_ih[:, 2 * H + c * S : 2 * H + (c + 1) * S].T, dtype=f32)
    bhn = np.ascontiguousarray(b_hh[:, 2 * H + c * S : 2 * H + (c + 1) * S].T, dtype=f32)

    x2 = np.ascontiguousarray(input.reshape(2, 1), dtype=f32)
    wout = np.ascontiguousarray(W_out[0].reshape(KC, S).T, dtype=f32)  # [S, KC]
    boutA = np.ascontiguousarray(b_out.reshape(1, 1), dtype=f32)
    id8 = np.eye(KC, dtype=f32)

    return {
        "wt0": wt0, "wtih": wtih, "wthh": wthh, "hrhs": hrhs, "hown": hown,
        "brz": brz, "bin": bin_, "bhn": bhn, "x2": x2, "wout": wout,
        "bout": boutA, "id8": id8,
    }


_NC_CACHE = {}


def kernel(input, hiddens, W_ih0, W_ih, W_hh, b_ih, b_hh, W_out, b_out):
    args = (input, hiddens, W_ih0, W_ih, W_hh, b_ih, b_hh, W_out, b_out)
    args = [np.asarray(a, dtype=np.float32) for a in args]

    if "nc" not in _NC_CACHE:
        _NC_CACHE["nc"] = build_kernel()
    nc = _NC_CACHE["nc"]

    in_maps = [_prep_core_inputs(c, *args) for c in range(N_CORES)]
    res = run_bass_kernel_spmd(nc, in_maps, core_ids=list(range(N_CORES)))

    out0 = res.results[0]
    flow = out0["flow_out"].reshape(1, 1, 1).astype(np.float32)
    hidp = out0["hid_out"].reshape(L, 1, H).astype(np.float32)
    return flow, hidp


# revision 9
# speedup vs baseline: 1.6838x; 1.6838x over previous
"""8-layer GRU (H=1000, INPUT=2, batch=1, single step) on 8 trn2 NeuronCores.

Tensor-parallel row sharding: core c owns hidden units [c*125,(c+1)*125)
of every gate of every layer.  gh_l = W_hh[l] @ h_l depends only on the
input hidden states and streams/computes in parallel; gi_l = W_ih[l] @
h'_{l-1} chains through layers, with the full h'_{l-1} re-assembled by a
small per-layer AllGather (500B per rank, 7 of them — layer 7's shard
is returned per-core and unsharded on the host).  A dummy AllGather at
kernel start absorbs the first-collective rendezvous while weights
stream.

Layouts (host-prepared per core):
- Weight tiles [128, 8*375]: tile[p, kc*375 + g*125 + u] =
  W[g*1000 + c*125 + u, kc*125 + p], rows 125..127 zero-padded
  (128-partition DMAs run ~3x faster than 125-partition ones here, and
  zero pad rows make the extra contraction lanes harmless).
- Matvec: out[1,375] += h_chunk[128,1].T @ Wtile[:, kc*375:(kc+1)*375].
  h is the stationary operand (LDWEIGHTS of one column is ~free);
  the weight slice is the moving operand at 1 column/cycle.
- Gate math runs on [1, *] row vectors on partition 0.
- The gathered h' ([8,125] rank-major) is PE-transposed via an identity
  to [125,8], the per-chunk rhs layout of the next layer's gi.
"""
import numpy as np

from concourse import bass, mybir, tile
from concourse.bass_utils import run_bass_kernel_spmd

N_CORES = 8
L = 8          # layers
H = 1000
S = 125        # hidden shard per core
KC = 8         # contraction chunks
G = 3          # gates r,z,n
P = 128        # padded partition dim
GS = G * S     # 375
FP = mybir.dt.float32

# ---------------------------------------------------------------------------
# workaround: this container's walrus build rejects >1 sync wait per
# instruction; split extras onto same-engine carrier NOPs.
MAX_WAITS_PER_INST = 1


def _split_multi_waits(nc, max_waits: int = MAX_WAITS_PER_INST) -> int:
    n_split = 0
    for f in nc.m.functions:
        for bb in f.blocks:
            insts = bb.instructions
            out = []
            for inst in insts:
                si = inst.sync_info
                if si is not None and len(si.on_wait) > max_waits:
                    waits = list(si.on_wait)
                    extra, keep = waits[:-max_waits], waits[-max_waits:]
                    for w in extra:
                        n_split += 1
                        nop = mybir.InstNoOp(
                            name=f"waitsplit-{n_split}-{inst.name}",
                            sync_info=mybir.SyncInfo(on_wait=[w], on_update=[]),
                            bass_nofuse=True,
                            engine=inst.engine,
                        )
                        out.append(nop)
                    si.on_wait = keep
                out.append(inst)
            insts[:] = out
    return n_split


# ---------------------------------------------------------------------------
def build_kernel() -> bass.Bass:
    nc = bass.Bass()

    wt0 = nc.declare_dram_parameter("wt0", [2, GS], FP, isOutput=False)
    wtih = nc.declare_dram_parameter("wtih", [L - 1, P, KC * GS], FP, isOutput=False)
    wthh = nc.declare_dram_parameter("wthh", [L, P, KC * GS], FP, isOutput=False)
    hrhs = nc.declare_dram_parameter("hrhs", [P, L * KC], FP, isOutput=False)
    hown = nc.declare_dram_parameter("hown", [1, L * S], FP, isOutput=False)
    brz = nc.declare_dram_parameter("brz", [1, L * 2 * S], FP, isOutput=False)
    bin_ = nc.declare_dram_parameter("bin", [1, L * S], FP, isOutput=False)
    bhn = nc.declare_dram_parameter("bhn", [1, L * S], FP, isOutput=False)
    x2 = nc.declare_dram_parameter("x2", [2, 1], FP, isOutput=False)
    wown = nc.declare_dram_parameter("wown", [1, S], FP, isOutput=False)
    bout8 = nc.declare_dram_parameter("bout8", [1, 1], FP, isOutput=False)
    id8 = nc.declare_dram_parameter("id8", [KC, KC], FP, isOutput=False)

    hid_sh = nc.declare_dram_parameter("hid_sh", [L, S], FP, isOutput=True)
    flow_pt = nc.declare_dram_parameter("flow_pt", [1, 1], FP, isOutput=True)

    with tile.TileContext(nc) as tc:
        with (
            tc.tile_pool(name="dram", bufs=1, space="DRAM") as dram,
            tc.tile_pool(name="wp", bufs=1) as wp,
            tc.tile_pool(name="small", bufs=1) as small,
            tc.tile_pool(name="tmp", bufs=2) as tmp,
            tc.tile_pool(name="ps", bufs=2, space="PSUM") as ps,
        ):
            # --- dummy collective: absorb first-collective rendezvous ---
            dummy_in = dram.tile([8, 1], FP, tag="dummy_in")
            dummy_out = dram.tile([64, 1], FP, tag="dummy_out")
            nc.gpsimd.collective_compute(
                "AllGather",
                mybir.AluOpType.bypass,
                replica_groups=[list(range(N_CORES))],
                ins=[dummy_in.opt()],
                outs=[dummy_out.opt()],
            )

            # --- small loads (sync HWDGE; off the gpsimd queue) ---
            x_sb = small.tile([2, 1], FP, tag="x")
            nc.sync.dma_start(out=x_sb[:], in_=x2[:])
            wt0_sb = small.tile([2, GS], FP, tag="wt0")
            nc.sync.dma_start(out=wt0_sb[:], in_=wt0[:])
            hrhs_sb = small.tile([P, L * KC], FP, tag="hrhs")
            nc.sync.dma_start(out=hrhs_sb[:], in_=hrhs[:])
            hown_sb = small.tile([1, L * S], FP, tag="hown")
            nc.sync.dma_start(out=hown_sb[:], in_=hown[:])
            brz_sb = small.tile([1, L * 2 * S], FP, tag="brz")
            nc.sync.dma_start(out=brz_sb[:], in_=brz[:])
            bin_sb = small.tile([1, L * S], FP, tag="bin")
            nc.sync.dma_start(out=bin_sb[:], in_=bin_[:])
            bhn_sb = small.tile([1, L * S], FP, tag="bhn")
            nc.sync.dma_start(out=bhn_sb[:], in_=bhn[:])
            wown_sb = small.tile([1, S], FP, tag="wown")
            nc.sync.dma_start(out=wown_sb[:], in_=wown[:])
            bout_sb = small.tile([1, 1], FP, tag="bout")
            nc.sync.dma_start(out=bout_sb[:], in_=bout8[:])
            id8_sb = small.tile([KC, KC], FP, tag="id8")
            nc.sync.dma_start(out=id8_sb[:], in_=id8[:])

            # --- weight DMAs (gpsimd SWDGE: spread across all DMA engines) ---
            wtih_sb = [None] * L
            wthh_sb = [None] * L
            wthh_sb[0] = wp.tile([P, KC * GS], FP, tag="whhX", name="whh0", bufs=2)
            nc.gpsimd.dma_start(out=wthh_sb[0][:], in_=wthh[0])
            for l in range(1, L):
                wtih_sb[l] = wp.tile(
                    [P, KC * GS], FP, name=f"wih{l}", bufs=2,
                    tag="wihX" if l % 2 == 1 else "wihY",
                )
                nc.gpsimd.dma_start(out=wtih_sb[l][:], in_=wtih[l - 1])
                wthh_sb[l] = wp.tile(
                    [P, KC * GS], FP, name=f"whh{l}", bufs=2,
                    tag="whhX" if l % 2 == 0 else "whhY",
                )
                nc.gpsimd.dma_start(out=wthh_sb[l][:], in_=wthh[l])

            hprhs = [None] * L
            hp_sb = None

            for l in range(L):
                # ---- gi -> gip [1, GS] ----
                gip = ps.tile([1, GS], FP, tag="gip")
                if l == 0:
                    nc.tensor.matmul(gip[:], x_sb[:], wt0_sb[:], start=True, stop=True)
                else:
                    for kc in range(KC):
                        nc.tensor.matmul(
                            gip[:],
                            hprhs[l - 1][:, kc : kc + 1],
                            wtih_sb[l][:, kc * GS : (kc + 1) * GS],
                            start=(kc == 0),
                            stop=(kc == KC - 1),
                        )
                # ---- gh -> ghp [1, GS] ----
                ghp = ps.tile([1, GS], FP, tag="ghp")
                for kc in range(KC):
                    nc.tensor.matmul(
                        ghp[:],
                        hrhs_sb[:, l * KC + kc : l * KC + kc + 1],
                        wthh_sb[l][:, kc * GS : (kc + 1) * GS],
                        start=(kc == 0),
                        stop=(kc == KC - 1),
                    )

                # ---- gate math on [1, *] rows ----
                ghsb = tmp.tile([1, GS], FP, tag="ghsb")
                nc.scalar.copy(out=ghsb[:], in_=ghp[:])
                # r|z = sigmoid(gi + gh + b)
                t_rz = tmp.tile([1, 2 * S], FP, tag="t_rz")
                nc.vector.tensor_add(t_rz[:], gip[0:1, 0 : 2 * S], ghsb[0:1, 0 : 2 * S])
                t_rz2 = tmp.tile([1, 2 * S], FP, tag="t_rz2")
                nc.vector.tensor_add(
                    t_rz2[:], t_rz[:], brz_sb[0:1, l * 2 * S : (l + 1) * 2 * S]
                )
                rz = tmp.tile([1, 2 * S], FP, tag="rz")
                nc.scalar.activation(
                    rz[:], t_rz2[:], mybir.ActivationFunctionType.Sigmoid
                )
                # n = tanh(gi_n + b_in + r * (gh_n + b_hn))
                hn = tmp.tile([1, S], FP, tag="hn")
                nc.vector.tensor_add(
                    hn[:], ghsb[0:1, 2 * S : GS], bhn_sb[0:1, l * S : (l + 1) * S]
                )
                rhn = tmp.tile([1, S], FP, tag="rhn")
                nc.vector.tensor_mul(rhn[:], rz[0:1, 0:S], hn[:])
                ginb = tmp.tile([1, S], FP, tag="ginb")
                nc.vector.tensor_add(
                    ginb[:], gip[0:1, 2 * S : GS], bin_sb[0:1, l * S : (l + 1) * S]
                )
                s_n = tmp.tile([1, S], FP, tag="s_n")
                nc.vector.tensor_add(s_n[:], ginb[:], rhn[:])
                n_sb = tmp.tile([1, S], FP, tag="n")
                nc.scalar.activation(n_sb[:], s_n[:], mybir.ActivationFunctionType.Tanh)
                # h' = n + z * (h - n)
                d_sb = tmp.tile([1, S], FP, tag="d")
                nc.vector.tensor_sub(
                    d_sb[:], hown_sb[0:1, l * S : (l + 1) * S], n_sb[:]
                )
                zd = tmp.tile([1, S], FP, tag="zd")
                nc.vector.tensor_mul(zd[:], rz[0:1, S : 2 * S], d_sb[:])
                hp_sb = tmp.tile([1, S], FP, tag="hp", name=f"hp{l}", bufs=3)
                nc.vector.tensor_add(hp_sb[:], zd[:], n_sb[:])

                # own-shard output (sync engine, off critical path)
                nc.sync.dma_start(out=hid_sh[l : l + 1, :], in_=hp_sb[:])

                if l < L - 1:
                    # ---- exchange ----
                    ag_in = dram.tile([1, S], FP, tag=f"agin{l}", name=f"agin{l}")
                    nc.scalar.dma_start(out=ag_in[:], in_=hp_sb[:])
                    ag_out = dram.tile([KC, S], FP, tag=f"agout{l}", name=f"agout{l}")
                    nc.gpsimd.collective_compute(
                        "AllGather",
                        mybir.AluOpType.bypass,
                        replica_groups=[list(range(N_CORES))],
                        ins=[ag_in.opt()],
                        outs=[ag_out.opt()],
                    )
                    hpnat = tmp.tile([KC, S], FP, tag="hpnat")
                    nc.scalar.dma_start(out=hpnat[:], in_=ag_out[:])
                    trp = ps.tile([S, KC], FP, tag="trp")
                    nc.tensor.matmul(trp[:], hpnat[:], id8_sb[:], start=True, stop=True)
                    hprhs[l] = small.tile([P, KC], FP, tag=f"hprhs{l}", name=f"hprhs{l}")
                    nc.vector.memset(hprhs[l][:], 0.0)
                    nc.vector.tensor_copy(out=hprhs[l][0:S, :], in_=trp[:])

            # ---- flow partial: dot(wout_own, hp7) + b_out/8 ----
            prod = tmp.tile([1, S], FP, tag="prod")
            nc.vector.tensor_mul(prod[:], wown_sb[:], hp_sb[:])
            red = tmp.tile([1, 1], FP, tag="red")
            nc.vector.tensor_reduce(
                out=red[:], in_=prod[:], op=mybir.AluOpType.add,
                axis=mybir.AxisListType.X,
            )
            flow_sb = tmp.tile([1, 1], FP, tag="flow")
            nc.vector.tensor_add(flow_sb[:], red[:], bout_sb[:])
            nc.sync.dma_start(out=flow_pt[:], in_=flow_sb[:])

    _split_multi_waits(nc)
    return nc


# ---------------------------------------------------------------------------
def _prep_core_inputs(c, input, hiddens, W_ih0, W_ih, W_hh, b_ih, b_hh, W_out, b_out):
    f32 = np.float32
    rows = np.concatenate(
        [np.arange(g * H + c * S, g * H + (c + 1) * S) for g in range(G)]
    )

    def wtile(W):  # [3H, H] -> [P, KC*GS], zero-padded rows 125..127
        R = W[rows, :]                                   # [GS, H]
        T = R.T.reshape(KC, S, GS)                       # [KC, S(k), GS]
        A = np.zeros((P, KC, GS), f32)
        A[:S] = T.transpose(1, 0, 2)
        return np.ascontiguousarray(A.reshape(P, KC * GS))

    wt0 = np.ascontiguousarray(W_ih0[rows, :].T, dtype=f32)       # [2, GS]
    wtih = np.stack([wtile(W_ih[i]) for i in range(L - 1)])
    wthh = np.stack([wtile(W_hh[i]) for i in range(L)])

    hmat = hiddens[:, 0, :]                                       # [L, H]
    hr = np.zeros((P, L * KC), f32)
    hr[:S] = hmat.reshape(L, KC, S).transpose(2, 0, 1).reshape(S, L * KC)
    hown = np.ascontiguousarray(
        hmat[:, c * S : (c + 1) * S].reshape(1, L * S), dtype=f32
    )

    bsum = (b_ih + b_hh).astype(f32)                              # [L, 3H]
    brz = np.empty((1, L * 2 * S), f32)
    for l in range(L):
        brz[0, l * 2 * S : l * 2 * S + S] = bsum[l, c * S : (c + 1) * S]
        brz[0, l * 2 * S + S : (l + 1) * 2 * S] = bsum[l, H + c * S : H + (c + 1) * S]
    bin_ = np.ascontiguousarray(
        b_ih[:, 2 * H + c * S : 2 * H + (c + 1) * S].reshape(1, L * S), dtype=f32
    )
    bhn = np.ascontiguousarray(
        b_hh[:, 2 * H + c * S : 2 * H + (c + 1) * S].reshape(1, L * S), dtype=f32
    )

    x2 = np.ascontiguousarray(input.reshape(2, 1), dtype=f32)
    wown = np.ascontiguousarray(W_out[0, c * S : (c + 1) * S].reshape(1, S), dtype=f32)
    bout8 = np.full((1, 1), float(np.asarray(b_out).reshape(-1)[0]) / N_CORES, f32)
    id8 = np.eye(KC, dtype=f32)

    return {
        "wt0": wt0, "wtih": wtih, "wthh": wthh, "hrhs": hr, "hown": hown,
        "brz": brz, "bin": bin_, "bhn": bhn, "x2": x2, "wown": wown,
        "bout8": bout8, "id8": id8,
    }


_NC_CACHE = {}


def kernel(input, hiddens, W_ih0, W_ih, W_hh, b_ih, b_hh, W_out, b_out):
    args = (input, hiddens, W_ih0, W_ih, W_hh, b_ih, b_hh, W_out, b_out)
    args = [np.asarray(a, dtype=np.float32) for a in args]

    if "nc" not in _NC_CACHE:
        _NC_CACHE["nc"] = build_kernel()
    nc = _NC_CACHE["nc"]

    in_maps = [_prep_core_inputs(c, *args) for c in range(N_CORES)]
    res = run_bass_kernel_spmd(nc, in_maps, core_ids=list(range(N_CORES)))

    hid = np.empty((L, H), np.float32)
    flow = np.float64(0.0)
    for c in range(N_CORES):
        out_c = res.results[c]
        hid[:, c * S : (c + 1) * S] = out_c["hid_sh"]
        flow += np.float64(out_c["flow_pt"][0, 0])
    return (
        np.float32(flow).reshape(1, 1, 1),
        hid.reshape(L, 1, H).astype(np.float32),
    )
